# revision 36
# baseline (speedup 1.0000x reference)
"""EntityAwareAttention TRN2 Bass kernel — 8-core data parallel.

Problem (per full batch): B=64, L=256, H=1024, P=64, A=512, T=8.
  e1_h/e2_h   = word_hiddens gathered at e1_end/e2_end           [B, H]
  e*_type     = softmax(e_h @ tE.T) @ tE                          [B, H]
  ef          = concat(e1_h, e1_type, e2_h, e2_type)              [B, 4H]
  dense_pos   = concat(wh, pos_e1, pos_e2) @ W_pos                [B, L, A]
  dense_ent   = ef @ W_ent                                        [B, A]
  u           = tanh(dense_pos + repeat-interleave(dense_ent))    [B, L, A]
                (addend for (l, a) is dense_ent[b, 2l + (a>=256)])
  vu          = u @ v ; alpha = softmax(vu, axis=L)               [B, L]
  z           = sum_l alpha[b,l] * wh[b,l,:]                      [B, H]

Sharding: batch across 8 cores (8 batches/core); weights replicated.
The host additionally ships pos_featuresT = concat(wh, p1, p2).T per core
(pure layout prep, no arithmetic) so the contraction dim is already on
partitions for the big matmul.

Per-core structure:
  * tokens t = b*L + l; 16 token-tiles of 128.
  * dense_pos[t_tile] = sum_k whT_k.T @ W_pos_k  (9 fp32r matmuls/tile at
    full PE rate, N=512). fp32r-typed DRAM tensors carry plain fp32 bytes;
    the dtype satisfies the fp32r-producer verifier with no cast ops.
  * dense_ent addend applied as per-partition ACT bias in the tanh
    (halves a<256 / a>=256) from a transposed parity-split dense_ent.
  * vu via DVE mult + reduce; softmax over L through tiny PE transposes
    into an [8, 256] batch-major view.
  * z via block-diagonal alpha: z[8, A-chunk] += alpha_blocked_i.T @ wh_i
    accumulated across all 16 token-tiles in PSUM (zero columns mask
    foreign batches).
"""

import numpy as np
import ml_dtypes

import concourse.bass as bass
import concourse.tile as tile
from concourse import bacc, mybir
from concourse.bass_utils import run_bass_kernel_spmd

F32 = mybir.dt.float32
F32R = mybir.dt.float32r
BF16 = mybir.dt.bfloat16
I32 = mybir.dt.int32
AF = mybir.ActivationFunctionType
ALU = mybir.AluOpType

B, L, H, P2, A, T = 64, 256, 1024, 64, 512, 8
NCORES = 8
BL = B // NCORES            # 8 local batches
TOK = BL * L                # 2048 tokens
NT = TOK // 128             # 16 token tiles
F = H + 2 * P2              # 1152 contraction dim
KF = F // 128               # 9 k-tiles
KE = 4 * H // 128           # 32 W_ent k-tiles
HC = H // 128               # 8 h-chunks


def _build_core(tc):
    nc = tc.nc
    whT_d = nc.dram_tensor("whT", [F, TOK], BF16, kind="ExternalInput").ap()
    wh_d = nc.dram_tensor("word_hiddens", [TOK, H], F32R, kind="ExternalInput").ap()
    whz_d = nc.dram_tensor("wh_bf16", [TOK, H], BF16, kind="ExternalInput").ap()
    e1_d = nc.dram_tensor("e1_end", [BL, 1], I32, kind="ExternalInput").ap()
    e2_d = nc.dram_tensor("e2_end", [BL, 1], I32, kind="ExternalInput").ap()
    te_d = nc.dram_tensor("type_embeddings", [T, H], BF16, kind="ExternalInput").ap()
    wpos_d = nc.dram_tensor("W_pos", [F, A], BF16, kind="ExternalInput").ap()
    went_d = nc.dram_tensor("W_ent", [4 * H, A], BF16, kind="ExternalInput").ap()
    v_d = nc.dram_tensor("v", [1, A], F32, kind="ExternalInput").ap()
    out_d = nc.dram_tensor("out", [BL, H], F32, kind="ExternalOutput").ap()

    const = tc.alloc_tile_pool(name="const", bufs=1)
    whs = tc.alloc_tile_pool(name="whs", bufs=4)
    work = tc.alloc_tile_pool(name="work", bufs=2)
    went_pool = tc.alloc_tile_pool(name="went", bufs=6)
    ps_dp = tc.alloc_tile_pool(name="ps_dp", bufs=3, space="PSUM")
    ps_tr = tc.alloc_tile_pool(name="ps_tr", bufs=2, space="PSUM")
    ps_sm = tc.alloc_tile_pool(name="ps_sm", bufs=3, space="PSUM")

    # ---- gather chain first (Pool FIFO must reach the indirect DMA fast) ----
    ends = const.tile([2 * BL, 1], I32)
    nc.sync.dma_start(ends[0:BL, :], e1_d[:])
    nc.sync.dma_start(ends[BL:2 * BL, :], e2_d[:])
    gidx = const.tile([2 * BL, 1], I32)
    nc.gpsimd.iota(gidx[:], pattern=[[1, 1]], base=0, channel_multiplier=L)
    nc.vector.tensor_scalar(out=gidx[:], in0=gidx[:], scalar1=BL * L - 1,
                            scalar2=None, op0=ALU.bitwise_and)
    nc.vector.tensor_tensor(out=gidx[:], in0=gidx[:], in1=ends[:], op=ALU.add)

    eh = const.tile([2 * BL, H], F32R)
    nc.gpsimd.indirect_dma_start(
        out=eh[:], out_offset=None, in_=wh_d[:],
        in_offset=bass.IndirectOffsetOnAxis(ap=gidx[:, 0:1], axis=0))

    # ---- other constants / small loads ----
    iota_p = const.tile([128, 128], I32)
    iota_f = const.tile([128, 128], I32)
    nc.gpsimd.iota(iota_p[:], pattern=[[0, 128]], base=0, channel_multiplier=1)
    nc.gpsimd.iota(iota_f[:], pattern=[[1, 128]], base=0, channel_multiplier=0)
    ident = const.tile([128, 128], F32R)
    nc.vector.tensor_tensor(out=ident[:], in0=iota_p[:], in1=iota_f[:],
                            op=ALU.is_equal)

    ones = const.tile([128, 1], F32)
    nc.gpsimd.memset(ones[:], 1.0)
    ident_bf = const.tile([128, 128], BF16)
    nc.vector.tensor_copy(out=ident_bf[:], in_=ident[:].bitcast(F32))

    v_sb = const.tile([1, A], F32)
    nc.sync.dma_start(v_sb[:], v_d[:])
    v_bc = const.tile([128, A], F32)
    nc.gpsimd.partition_broadcast(v_bc[:], v_sb[0:1, :])

    te_sb = const.tile([T, H], BF16)
    nc.sync.dma_start(te_sb[:], te_d[:])

    # ---- first dense_pos operands, then W_ent stream ----
    wpos = const.tile([128, KF * A], BF16)
    nc.sync.dma_start(
        wpos.rearrange("p (k a) -> p k a", k=KF)[:, 0:3],
        wpos_d.rearrange("(k p) a -> p k a", p=128)[:, 0:3])
    whT = const.tile([128, NT * F], BF16)
    whT_cols = whT_d.rearrange("(k p) t -> p k t", p=128)
    nc.sync.dma_start(
        whT.rearrange("p (j k c) -> p j k c", j=NT // 2, k=KF)[:, 0],
        whT_cols[:, :, 0:256])

    # ---- W_ent stream (dense_ent gates the tanh bias) ----
    went_tiles = []
    for q in range(KE // 4):
        wt = went_pool.tile([128, 4 * A], BF16, tag="went", name=f"went{q}")
        nc.sync.dma_start(
            wt.rearrange("p (g a) -> p g a", g=4),
            went_d[q * 512:(q + 1) * 512, :].rearrange("(g p) a -> p g a", p=128))
        went_tiles.append(wt)

    # ---- big constant loads (whT block 0 + wpos k0..2 were loaded above) ----
    for g in range(1, 3):
        nc.sync.dma_start(
            wpos.rearrange("p (k a) -> p k a", k=KF)[:, g * 3:(g + 1) * 3],
            wpos_d.rearrange("(k p) a -> p k a", p=128)[:, g * 3:(g + 1) * 3])
    wh_sb = []
    for j in range(NT // 2):
        if j > 0:
            nc.sync.dma_start(
                whT.rearrange("p (j k c) -> p j k c", j=NT // 2, k=KF)[:, j],
                whT_cols[:, :, j * 256:(j + 1) * 256])
        for i in (2 * j, 2 * j + 1):
            wt = whs.tile([128, H], BF16, tag="wh", name=f"wh{i}")
            nc.sync.dma_start(wt[:], whz_d[i * 128:(i + 1) * 128, :])
            wh_sb.append(wt)

    # ---- hoisted dense_pos matmuls for tiles 0-1 (fill the PE head while
    # the gather/entity chain is still in flight) ----
    dp_pre = []
    for i in range(2):
        dp = ps_dp.tile([128, A], F32, tag="dp", name=f"dp_pre{i}")
        for k in range(KF):
            off = (i // 2) * KF * 256 + k * 256 + (i % 2) * 128
            nc.tensor.matmul(
                dp[:], lhsT=whT[:, off: off + 128],
                rhs=wpos[:, k * A:(k + 1) * A],
                start=(k == 0), stop=(k == KF - 1))
        dp_pre.append(dp)

    teT = const.tile([128, HC * T], BF16)
    for hc in range(HC):
        pt = ps_tr.tile([128, 128], F32R, tag="tr")
        ptb = pt.bitcast(BF16)
        nc.tensor.transpose(ptb[:, 0:T], te_sb[:, hc * 128:(hc + 1) * 128],
                            ident_bf[0:T, 0:T])
        nc.vector.tensor_copy(out=teT[:, hc * T:(hc + 1) * T], in_=ptb[:, 0:T])

    # ---- entity features efT[:, kt*8:+8], 32 k-tiles ----
    # regions: 0=e1_h(kt0..7) 1=e2_h(8..15) 2=e1_type(16..23) 3=e2_type(24..31)
    # (W_ent rows are host-permuted to match, so dense_ent can consume the
    # gather-only e_h halves before the softmax chain finishes.)
    efT = const.tile([128, KE * BL], BF16)
    for hc in range(HC):
        pt = ps_tr.tile([128, 128], F32R, tag="tr")
        nc.tensor.transpose(pt[:, 0:2 * BL], eh[:, hc * 128:(hc + 1) * 128],
                            ident[0:2 * BL, 0:2 * BL])
        nc.vector.tensor_copy(out=efT[:, hc * BL:(hc + 1) * BL],
                              in_=pt[:, 0:BL].bitcast(F32))
        nc.vector.tensor_copy(
            out=efT[:, (HC + hc) * BL:(HC + hc + 1) * BL],
            in_=pt[:, BL:2 * BL].bitcast(F32))

    for ent in range(2):
        sc = ps_sm.tile([BL, T], F32, tag="sm")
        for hc in range(HC):
            col = (0 if ent == 0 else HC) + hc
            nc.tensor.matmul(sc[:], lhsT=efT[:, col * BL:(col + 1) * BL],
                             rhs=teT[:, hc * T:(hc + 1) * T],
                             start=(hc == 0), stop=(hc == HC - 1))
        asm = const.tile([BL, T], F32, tag=f"asm{ent}")
        ssum = const.tile([BL, 1], F32, tag=f"ssum{ent}")
        nc.scalar.activation(asm[:], sc[:], AF.Exp, accum_out=ssum[:])
        rs = const.tile([BL, 1], F32, tag=f"rs{ent}")
        nc.vector.reciprocal(rs[:], ssum[:])
        al = const.tile([BL, T], F32R, tag=f"al{ent}")
        nc.vector.tensor_scalar(out=al[:], in0=asm[:], scalar1=rs[:, 0:1],
                                scalar2=None, op0=ALU.mult)
        pt = ps_tr.tile([128, 128], F32R, tag="tr")
        nc.tensor.transpose(pt[0:T, 0:BL], al[:],
                            ident[0:BL, 0:BL])
        alTe = const.tile([T, BL], BF16, tag=f"alTe{ent}")
        nc.vector.tensor_copy(out=alTe[:], in_=pt[0:T, 0:BL].bitcast(F32))
        for hc in range(HC):
            pe = ps_sm.tile([128, BL], F32, tag="sm")
            nc.tensor.matmul(pe[:], lhsT=te_sb[:, hc * 128:(hc + 1) * 128],
                             rhs=alTe[:], start=True, stop=True)
            col = (2 * HC if ent == 0 else 3 * HC) + hc
            nc.vector.tensor_copy(out=efT[:, col * BL:(col + 1) * BL],
                                  in_=pe[:])

    # ---- dense_ent matmuls (W_ent tiles DMA'd up front) ----
    de = ps_sm.tile([BL, A], F32, tag="sm")
    for k in range(KE):
        nc.tensor.matmul(de[:], lhsT=efT[:, k * BL:(k + 1) * BL],
                         rhs=went_tiles[k // 4][:, (k % 4) * A:(k % 4 + 1) * A],
                         start=(k == 0), stop=(k == KE - 1))

    # parity split (even a's then odd a's) + transpose to [l(p), b] bias cols
    de_eo = const.tile([BL, A], F32R)
    nc.vector.tensor_copy(
        out=de_eo.rearrange("b (two l) -> b two l", two=2),
        in_=de.rearrange("b (l two) -> b two l", two=2))
    # bias_sb cols: parity*16 + half*8 + b
    bias_sb = const.tile([128, 32], F32)
    for par in range(2):
        for half in range(2):
            pt = ps_tr.tile([128, 128], F32R, tag="tr")
            src = de_eo[:, par * 256 + half * 128: par * 256 + (half + 1) * 128]
            nc.tensor.transpose(pt[:, 0:BL], src, ident[0:BL, 0:BL])
            nc.vector.tensor_copy(
                out=bias_sb[:, par * 16 + half * 8: par * 16 + half * 8 + BL],
                in_=pt[:, 0:BL].bitcast(F32))

    # ---- main loop over token tiles ----
    # Unnormalized attention: w = exp(vu) accumulates into z immediately
    # (block-diagonal matmul); normalization by 1/sum(exp) happens once at
    # the end. exp args are bounded (|vu| <= sum|v| ~ 25) so no max-shift.
    vu0 = const.tile([128, BL], F32)     # vu for l in [0,128), col = b
    vu1 = const.tile([128, BL], F32)     # vu for l in [128,256), col = b
    expc0 = const.tile([128, BL], F32)   # exp(vu) same layout
    expc1 = const.tile([128, BL], F32)
    alblk = const.tile([128, NT * BL], BF16)
    nc.gpsimd.memset(alblk[:], 0.0)
    zp0 = ps_sm.tile([BL, A], F32, tag="sm")
    zp1 = ps_sm.tile([BL, A], F32, tag="sm")
    for i in range(NT):
        b, half = i // 2, i % 2
        if i < 2:
            dp = dp_pre[i]
        else:
            dp = ps_dp.tile([128, A], F32, tag="dp")
            for k in range(KF):
                off = (i // 2) * KF * 256 + k * 256 + (i % 2) * 128
                nc.tensor.matmul(
                    dp[:],
                    lhsT=whT[:, off: off + 128],
                    rhs=wpos[:, k * A:(k + 1) * A],
                    start=(k == 0), stop=(k == KF - 1))
        u = work.tile([128, A], F32, tag="u")
        nc.scalar.activation(u[:, 0:256], dp[:, 0:256], AF.Tanh,
                             bias=bias_sb[:, half * 8 + b: half * 8 + b + 1])
        nc.scalar.activation(u[:, 256:512], dp[:, 256:512], AF.Tanh,
                             bias=bias_sb[:, 16 + half * 8 + b: 16 + half * 8 + b + 1])
        scr = work.tile([128, A], F32, tag="scr")
        vu_dst = (vu0 if half == 0 else vu1)
        nc.vector.tensor_tensor(out=scr[:], in0=u[:], in1=v_bc[:], op=ALU.mult)
        nc.vector.tensor_reduce(out=vu_dst[:, b:b + 1], in_=scr[:],
                                axis=mybir.AxisListType.X, op=ALU.add)
        exp_dst = (expc0 if half == 0 else expc1)
        nc.scalar.activation(exp_dst[:, b:b + 1], vu_dst[:, b:b + 1], AF.Exp)
        nc.vector.tensor_copy(out=alblk[:, i * BL + b: i * BL + b + 1],
                              in_=exp_dst[:, b:b + 1])
        # z matmuls are deferred by one iteration: tile i-1's z runs after
        # tile i's dense_pos matmuls so the PE never stalls on the
        # tanh->vu->exp chain of the tile it just produced.
        if i > 0:
            for chunk, zp in ((0, zp0), (1, zp1)):
                nc.tensor.matmul(zp[:],
                                 lhsT=alblk[:, (i - 1) * BL:i * BL],
                                 rhs=wh_sb[i - 1][:, chunk * A:(chunk + 1) * A],
                                 start=(i == 1), stop=False)

    for chunk, zp in ((0, zp0), (1, zp1)):
        nc.tensor.matmul(zp[:],
                         lhsT=alblk[:, (NT - 1) * BL:NT * BL],
                         rhs=wh_sb[NT - 1][:, chunk * A:(chunk + 1) * A],
                         start=False, stop=True)

    # ---- normalization epilogue: esum via ones-matmul ----
    ecs = const.tile([128, BL], F32)
    nc.vector.tensor_tensor(out=ecs[:], in0=expc0[:], in1=expc1[:], op=ALU.add)
    esp = ps_tr.tile([128, 128], F32, tag="tr")
    nc.tensor.matmul(esp[0:BL, 0:1], lhsT=ecs[:], rhs=ones[:],
                     start=True, stop=True)
    ers = const.tile([BL, 1], F32)
    nc.vector.reciprocal(ers[:], esp[0:BL, 0:1])
    z_sb = const.tile([BL, H], F32)
    nc.scalar.activation(z_sb[:, 0:A], zp0[:], AF.Copy, scale=ers[:, 0:1])
    nc.scalar.activation(z_sb[:, A:H], zp1[:], AF.Copy, scale=ers[:, 0:1])

    nc.sync.dma_start(out_d[:], z_sb[:])

    for p in (ps_sm, ps_tr, ps_dp, went_pool, work, whs, const):
        p.release()


def build():
    nc = bacc.Bacc("TRN2", target_bir_lowering=False, debug=False,
                   num_devices=NCORES)
    with tile.TileContext(nc) as tc:
        _build_core(tc)
    nc.compile()
    return nc


_NC = None


def kernel(word_hiddens, pos_e1_embeddings, pos_e2_embeddings, e1_end, e2_end,
           type_embeddings, W_pos, W_ent, v):
    global _NC
    if _NC is None:
        _NC = build()
    wh = np.ascontiguousarray(word_hiddens, dtype=np.float32).reshape(B, L, H)
    p1 = np.ascontiguousarray(pos_e1_embeddings, dtype=np.float32).reshape(B, L, P2)
    p2 = np.ascontiguousarray(pos_e2_embeddings, dtype=np.float32).reshape(B, L, P2)
    e1 = np.asarray(e1_end, dtype=np.int32).reshape(B)
    e2 = np.asarray(e2_end, dtype=np.int32).reshape(B)
    te = np.ascontiguousarray(type_embeddings, dtype=np.float32).astype(ml_dtypes.bfloat16)
    wp = np.ascontiguousarray(W_pos, dtype=np.float32).astype(ml_dtypes.bfloat16)
    we0 = np.asarray(W_ent, dtype=np.float32).reshape(4, H, A)
    we = np.ascontiguousarray(
        np.concatenate([we0[0], we0[2], we0[1], we0[3]],
                       axis=0)).astype(ml_dtypes.bfloat16)
    vv = np.ascontiguousarray(v, dtype=np.float32).reshape(1, A)

    in_maps = []
    for c in range(NCORES):
        s = slice(c * BL, (c + 1) * BL)
        whc = np.ascontiguousarray(wh[s].reshape(TOK, H))
        pf = np.empty((TOK, F), dtype=np.float32)
        pf[:, :H] = whc
        pf[:, H:H + P2] = p1[s].reshape(TOK, P2)
        pf[:, H + P2:] = p2[s].reshape(TOK, P2)
        in_maps.append({
            "whT": np.ascontiguousarray(pf.T).astype(ml_dtypes.bfloat16),
            "word_hiddens": whc,
            "wh_bf16": whc.astype(ml_dtypes.bfloat16),
            "e1_end": e1[s].reshape(BL, 1),
            "e2_end": e2[s].reshape(BL, 1),
            "type_embeddings": te,
            "W_pos": wp,
            "W_ent": we,
            "v": vv,
        })
    res = run_bass_kernel_spmd(_NC, in_maps, core_ids=list(range(NCORES)))
    return np.concatenate([res.results[c]["out"] for c in range(NCORES)], axis=0)
